# revision 1
# baseline (speedup 1.0000x reference)
"""GQA attention (B=2, S=2048, H=2048, 32 heads / 8 KV groups, rope, causal-masked
softmax, output projection) distributed over 8 Trainium2 NeuronCores.

Sharding: data parallel over batch (2) x tensor parallel over KV groups (4 group-pairs).
Core c handles batch c//4 and KV groups {2*(c%4), 2*(c%4)+1} (= 8 q heads). Each core
computes its partial output projection (attn_out_shard @ wo_cols_shard.T); the host
sums the 4 partials per batch (the "all-reduce") and adds bo.

On-core layout trick: head dims of Q/K are permuted per head to [evens | odds] so rope
becomes block elementwise ops; scores are computed transposed (s_k on partitions) so
softmax denominators come free as an extra ones-column in V and attn@V feeds the
output projection without transposes. Matmuls run in float32r (full-rate, ~1.5e-4).
"""
import sys

for _p in ("/opt/trn_rl_repo",):
    if _p not in sys.path:
        sys.path.append(_p)

import numpy as np

S = 2048
H = 2048
HD = 64
NQT = 4          # s_q tiles of 512
NKT = 16         # s_k tiles of 128

_CACHE = {}


def _build(mode, has_bq, has_bk, has_bv):
    import concourse.bass as bass  # noqa: F401
    import concourse.mybir as mybir
    import concourse.tile as tile
    from concourse import bacc
    from concourse.masks import make_identity

    f32 = mybir.dt.float32
    f32r = mybir.dt.float32r
    AF = mybir.ActivationFunctionType
    ALU = mybir.AluOpType

    nc = bacc.Bacc("TRN2", target_bir_lowering=False, debug=False)
    xT = nc.dram_tensor("xT", [H, S], f32r, kind="ExternalInput")
    wqT = nc.dram_tensor("wqT", [H, 512], f32r, kind="ExternalInput")
    wkvT = nc.dram_tensor("wkvT", [H, 256], f32r, kind="ExternalInput")
    woR = nc.dram_tensor("woR", [512, H], f32r, kind="ExternalInput")
    COSd = nc.dram_tensor("COSx", [128, S], f32, kind="ExternalInput")
    SINd = nc.dram_tensor("SINx", [128, S], f32, kind="ExternalInput")
    outd = nc.dram_tensor("out", [S, H], f32, kind="ExternalOutput")
    m01d = nc.dram_tensor("m01", [128, 2048], f32, kind="ExternalInput") if mode == "causal" else None
    maskd = nc.dram_tensor("maskT", [S, S], f32, kind="ExternalInput") if mode == "generic" else None
    bqd = nc.dram_tensor("bq", [512, 1], f32, kind="ExternalInput") if has_bq else None
    bkvd = nc.dram_tensor("bkv", [256, 1], f32, kind="ExternalInput") if (has_bk or has_bv) else None

    with tile.TileContext(nc) as tc:
        with (
            tc.tile_pool(name="const", bufs=1) as cstp,
            tc.tile_pool(name="wts", bufs=1) as wts,
            tc.tile_pool(name="xs", bufs=3) as xsp,
            tc.tile_pool(name="per", bufs=1) as per,
            tc.tile_pool(name="rtmp", bufs=2) as rtp,
            tc.tile_pool(name="et", bufs=2) as etp,
            tc.tile_pool(name="rcs", bufs=2) as rcp,
            tc.tile_pool(name="outs", bufs=2) as outp,
            tc.tile_pool(name="mks", bufs=2) as mkp,
        ):
            COS = cstp.tile([128, S], f32, tag="cos")
            SIN = cstp.tile([128, S], f32, tag="sin")
            nc.sync.dma_start(COS[:], COSd[:])
            nc.sync.dma_start(SIN[:], SINd[:])
            ident = cstp.tile([128, 128], f32, tag="ident")
            make_identity(nc, ident[:])
            if mode == "causal":
                M01 = cstp.tile([128, 2048], f32, tag="m01")
                nc.sync.dma_start(M01[:], m01d[:])
            # bias tiles: partition dim max 128 -> split loads
            if has_bq:
                bq_t = [cstp.tile([128, 1], f32, tag=f"bq{m}", name=f"bq_t{m}") for m in range(4)]
                for m in range(4):
                    nc.sync.dma_start(bq_t[m][:], bqd[128 * m:128 * (m + 1), :])
            if has_bk or has_bv:
                bk_t = cstp.tile([128, 1], f32, tag="bkt")
                bv_t = cstp.tile([128, 1], f32, tag="bvt")
                nc.sync.dma_start(bk_t[:], bkvd[0:128, :])
                nc.sync.dma_start(bv_t[:], bkvd[128:256, :])

            # resident weights
            wq_t = [wts.tile([128, 512], f32r, tag=f"wq{k}", name=f"wq_t{k}") for k in range(16)]
            wkv_t = [wts.tile([128, 256], f32r, tag=f"wkv{k}", name=f"wkv_t{k}") for k in range(16)]
            for k in range(16):
                nc.sync.dma_start(wq_t[k][:], wqT[128 * k:128 * (k + 1), :])
                nc.sync.dma_start(wkv_t[k][:], wkvT[128 * k:128 * (k + 1), :])

            # persistent intermediates
            QTrot = [per.tile([128, S], f32r, tag=f"qtrot{m}", name=f"QTrot{m}") for m in range(4)]
            KTrot = per.tile([128, S], f32r, tag="ktrot")
            # V with a ones column per kt-block: [g0 v64 | 1 | g1 v64 | 1] x 16 kt
            Vp = per.tile([128, 130 * NKT], f32r, tag="vp")
            # denominator staging: head (m,hloc) -> den[hloc] partition 32*m (bases must be 0/32/64/96)
            den = [per.tile([128, 512], f32, tag=f"den{h}", name=f"den{h}") for h in range(2)]
            ones1 = per.tile([1, 512], f32, tag="ones1")
            nc.gpsimd.memset(ones1[:], 1.0)
            # selector tiles: sel[m] has ones in partition-row 32m; lhsT for the
            # K=128 matmul that broadcasts den row 32m across 64 psum partitions
            sel = [per.tile([128, 64], f32, tag=f"sel{m}", name=f"sel{m}") for m in range(4)]
            for m in range(4):
                nc.gpsimd.memset(sel[m][:], 0.0)
                nc.gpsimd.memset(sel[m][32 * m:32 * m + 1, :], 1.0)
            for h in range(2):
                nc.gpsimd.memset(den[h][:], 1.0)
            VTt = per.tile([128, 512], f32, tag="vtt")
            nc.gpsimd.memset(Vp[:].bitcast(f32), 1.0)  # ones columns at 130*kt+{64,129} survive
            # output-projection weights resident
            wo_t = [wts.tile([128, S], f32r, tag=f"wor{k}", name=f"wo_t{k}") for k in range(4)]
            for k in range(4):
                nc.sync.dma_start(wo_t[k][:], woR[128 * k:128 * (k + 1), :])

            # ---------- Phase P: projections + rope + V transpose ----------
            with tc.tile_pool(name="psP", bufs=1, space="PSUM") as psP:
                for sc in range(4):
                    ssl = slice(512 * sc, 512 * (sc + 1))
                    qp = [psP.tile([128, 512], f32, tag=f"qp{m}", name=f"qp{m}") for m in range(4)]
                    kvK = psP.tile([128, 512], f32, tag="kvK")
                    kvV = psP.tile([128, 512], f32, tag="kvV")
                    for k in range(16):
                        xk = xsp.tile([128, 512], f32r, tag="x")
                        nc.sync.dma_start(xk[:], xT[128 * k:128 * (k + 1), ssl])
                        st = (k == 0)
                        sp = (k == 15)
                        for m in range(4):
                            nc.tensor.matmul(qp[m][:], wq_t[k][:, 128 * m:128 * (m + 1)], xk[:], start=st, stop=sp)
                        nc.tensor.matmul(kvK[:], wkv_t[k][:, 0:128], xk[:], start=st, stop=sp)
                        nc.tensor.matmul(kvV[:], wkv_t[k][:, 128:256], xk[:], start=st, stop=sp)
                    if has_bq:
                        for m in range(4):
                            nc.vector.tensor_scalar_add(qp[m][:], qp[m][:], bq_t[m][:])
                    if has_bk:
                        nc.vector.tensor_scalar_add(kvK[:], kvK[:], bk_t[:])
                    # rope Q -> QTrot, K -> KTrot  (blocked layout [e|o] per head)
                    for src, dsts in [(qp, QTrot), ([kvK], [None])]:
                        for m, ps in enumerate(src):
                            dst = dsts[m] if dsts[m] is not None else KTrot
                            t1 = rtp.tile([128, 512], f32, tag="t1")
                            t2 = rtp.tile([128, 512], f32, tag="t2")
                            nc.vector.tensor_tensor(t1[:], ps[:], COS[:, ssl], ALU.mult)
                            for blk in range(4):
                                sb = 32 * (blk ^ 1)
                                db = 32 * blk
                                nc.vector.tensor_tensor(
                                    t2[db:db + 32, :], ps[sb:sb + 32, :], SIN[db:db + 32, ssl], ALU.mult)
                            nc.vector.tensor_tensor(dst[:, ssl], t1[:], t2[:], ALU.add)
                    # V: psum -> sbuf (+bias), transpose 128-blocks, scatter into Vp
                    if has_bv:
                        nc.vector.tensor_scalar_add(VTt[:], kvV[:], bv_t[:])
                    else:
                        nc.vector.tensor_copy(VTt[:], kvV[:])
                    for j in range(4):
                        kt = 4 * sc + j
                        vps = psP.tile([128, 128], f32, tag="vps", bufs=2)
                        nc.tensor.transpose(vps[:], VTt[:, 128 * j:128 * (j + 1)], ident[:])
                        # one copy: psum (128,(2,64)) -> Vp cols [130kt:+64] and [130kt+65:+129]
                        dst = Vp[:, 130 * kt:130 * kt + 130].rearrange("p (two x) -> p two x", two=2)[:, :, 0:64]
                        src_ap = vps[:].rearrange("p (two x) -> p two x", two=2)
                        nc.vector.tensor_copy(dst, src_ap)

            # ---------- Phase A+W: attention interleaved with output projection ----------
            with tc.tile_pool(name="psA", bufs=1, space="PSUM") as psA:
                for qt in range(NQT):
                    qsl = slice(512 * qt, 512 * (qt + 1))
                    n_kt = 4 * qt + 4 if mode == "causal" else NKT
                    for m in range(4):
                        # two heads (m: group 0 rows 0:64, m+4: group 1 rows 64:128) interleaved
                        av = [psA.tile([128, 512], f32, tag="av", bufs=2, name=f"av{h}") for h in range(2)]
                        for pr in range(n_kt // 2):
                            for hloc in range(2):
                                g = hloc
                                qb = 64 * hloc
                                sc2 = psA.tile([128, 1024], f32, tag="sc", bufs=2, name="sc2")
                                eT = etp.tile([128, 1024], f32r, tag="eT", name="eT")
                                for half in range(2):
                                    kt = 2 * pr + half
                                    nc.tensor.matmul(
                                        sc2[:, 512 * half:512 * (half + 1)],
                                        KTrot[64 * g:64 * g + 64, 128 * kt:128 * (kt + 1)],
                                        QTrot[m][qb:qb + 64, qsl],
                                        start=True, stop=True)
                                if mode == "generic":
                                    for half in range(2):
                                        kt = 2 * pr + half
                                        mk = mkp.tile([128, 512], f32, tag="mk", name="mk")
                                        nc.sync.dma_start(mk[:], maskd[128 * kt:128 * (kt + 1), qsl])
                                        stt = mkp.tile([128, 512], f32, tag="stt", name="stt")
                                        nc.vector.scalar_tensor_tensor(
                                            stt[:], sc2[:, 512 * half:512 * (half + 1)], 0.125, mk[:],
                                            ALU.mult, ALU.add)
                                        nc.scalar.activation(
                                            eT[:, 512 * half:512 * (half + 1)], stt[:], AF.Exp, scale=1.0)
                                else:
                                    nc.scalar.activation(eT[:], sc2[:], AF.Exp, scale=0.125)
                                if mode == "causal":
                                    for half in range(2):
                                        kt = 2 * pr + half
                                        t = kt - 4 * qt
                                        if 0 <= t <= 3:
                                            # only cols [0:128(t+1)] of this tile contain masked entries
                                            w = 128 * (t + 1)
                                            esl = slice(512 * half, 512 * half + w)
                                            nc.vector.tensor_tensor(
                                                eT[:, esl], eT[:, esl], M01[:, 512 * t:512 * t + w], ALU.mult)
                                for half in range(2):
                                    kt = 2 * pr + half
                                    nc.tensor.matmul(
                                        av[hloc][0:65, :], Vp[:, 130 * kt + 65 * g:130 * kt + 65 * g + 65],
                                        eT[:, 512 * half:512 * (half + 1)],
                                        start=(kt == 0), stop=(kt == n_kt - 1))
                        avq = per.tile([128, 512], f32r, tag=f"avtq{m}", bufs=2, name=f"avq{m}")
                        for hloc in range(2):
                            qb = 64 * hloc
                            # evict unnormalized AV and its denominator row; frees the psum bank
                            # (TensorCopy forbids partition-shifted out; tensor_scalar/DMA allow it)
                            nc.vector.tensor_scalar_mul(avq[qb:qb + 64, :], av[hloc][0:64, :], 1.0)
                            nc.vector.tensor_tensor(den[hloc][32 * m:32 * m + 1, :], av[hloc][64:65, :],
                                                    ones1[:], ALU.mult)
                        if m == 0:
                            avq_all = [avq]
                        else:
                            avq_all.append(avq)
                    # batched reciprocals: one op covers 4 heads (cost is free-size bound)
                    for hloc in range(2):
                        nc.vector.reciprocal(den[hloc][:], den[hloc][:])
                    for m in range(4):
                        # broadcast each head's recip row to 64 partitions via a selector matmul
                        for hloc in range(2):
                            rcb = psA.tile([128, 512], f32, tag="av", name="rcb", bufs=2)
                            nc.tensor.matmul(rcb[0:64, :], sel[m][:], den[hloc][:],
                                             start=True, stop=True)
                            qb = 64 * hloc
                            nc.vector.tensor_tensor(avq_all[m][qb:qb + 64, :],
                                                    avq_all[m][qb:qb + 64, :], rcb[0:64, :], ALU.mult)
                    # output projection for the 4 s-row-tiles of this qt block
                    for j in range(4):
                        mm = 4 * qt + j
                        for n in range(4):
                            nsl = slice(512 * n, 512 * (n + 1))
                            op = psA.tile([128, 512], f32, tag="op", bufs=2, name="op")
                            for k in range(4):
                                nc.tensor.matmul(op[:], avq_all[k][:, 128 * j:128 * (j + 1)],
                                                 wo_t[k][:, nsl], start=(k == 0), stop=(k == 3))
                            ot = outp.tile([128, 512], f32, tag="ot", name="ot")
                            nc.vector.tensor_copy(ot[:], op[:])
                            nc.sync.dma_start(outd[128 * mm:128 * (mm + 1), nsl], ot[:])

    nc.compile()
    return nc


_PERM64 = np.concatenate([np.arange(0, 64, 2), np.arange(1, 64, 2)])
# Q-tile m holds local heads (m, m+4) so each head's partition base (0/64) matches
# its KV group's base in KTrot (group g at rows 64g) - matmul requires equal bases.
_HEADS_ORDER = np.array([0, 4, 1, 5, 2, 6, 3, 7])


def _prep_core(c, x, freqs_cis, mask, wq, bq, wk, bk, wv, bv, wo, mode,
               has_bq, has_bk, has_bv):
    b, gp = divmod(c, 4)
    f = np.float32
    xT = np.ascontiguousarray(x[b].T, dtype=f)
    wq_c = wq[512 * gp:512 * (gp + 1)].reshape(8, 64, H)[_HEADS_ORDER][:, _PERM64, :].reshape(512, H)
    wqT = np.ascontiguousarray(wq_c.T, dtype=f)
    wk_c = wk[128 * gp:128 * (gp + 1)].reshape(2, 64, H)[:, _PERM64, :].reshape(128, H)
    wv_c = wv[128 * gp:128 * (gp + 1)]
    wkvT = np.ascontiguousarray(np.concatenate([wk_c, wv_c], 0).T, dtype=f)
    woR = wo[:, 512 * gp:512 * (gp + 1)].T.reshape(8, 64, H)[_HEADS_ORDER].reshape(512, H)
    woR = np.ascontiguousarray(woR, dtype=f)
    cosT = np.ascontiguousarray(freqs_cis[:, 0::2].T, dtype=f)   # (32, S)
    sinT = np.ascontiguousarray(freqs_cis[:, 1::2].T, dtype=f)
    COS = np.tile(cosT, (4, 1))
    SIN = np.concatenate([-sinT, sinT, -sinT, sinT], 0)
    m = {"xT": xT, "wqT": wqT, "wkvT": wkvT, "woR": woR,
         "COSx": np.ascontiguousarray(COS), "SINx": np.ascontiguousarray(SIN)}
    if mode == "causal":
        i = np.arange(128)[:, None]
        j = np.arange(512)[None, :]
        m["m01"] = np.concatenate([(j - i - 128 * t >= 0) for t in range(4)], axis=1).astype(f)
    if mode == "generic":
        m["maskT"] = np.ascontiguousarray(mask.T, dtype=f)
    if has_bq:
        bq_c = bq[512 * gp:512 * (gp + 1)].reshape(8, 64)[_HEADS_ORDER][:, _PERM64].reshape(512, 1)
        m["bq"] = np.ascontiguousarray(bq_c, dtype=f)
    if has_bk or has_bv:
        bk_c = bk[128 * gp:128 * (gp + 1)].reshape(2, 64)[:, _PERM64].reshape(128)
        bv_c = bv[128 * gp:128 * (gp + 1)]
        m["bkv"] = np.ascontiguousarray(np.concatenate([bk_c, bv_c]).reshape(256, 1), dtype=f)
    return m


def _detect_mode(mask):
    causal = np.where(np.tril(np.ones((S, S), dtype=bool)), np.float32(0.0), np.float32(-1e9))
    if np.array_equal(mask, causal):
        return "causal"
    if not np.any(mask):
        return "zeros"
    return "generic"


def _run(inputs, trace=False):
    from concourse import bass_utils
    x = np.asarray(inputs["x"], dtype=np.float32)
    freqs_cis = np.asarray(inputs["freqs_cis"], dtype=np.float32)
    mask = np.asarray(inputs["mask"], dtype=np.float32)
    wq = np.asarray(inputs["wq"], dtype=np.float32)
    bq = np.asarray(inputs["bq"], dtype=np.float32)
    wk = np.asarray(inputs["wk"], dtype=np.float32)
    bk = np.asarray(inputs["bk"], dtype=np.float32)
    wv = np.asarray(inputs["wv"], dtype=np.float32)
    bv = np.asarray(inputs["bv"], dtype=np.float32)
    wo = np.asarray(inputs["wo"], dtype=np.float32)
    bo = np.asarray(inputs["bo"], dtype=np.float32)

    mode = _detect_mode(mask)
    has_bq = bool(np.any(bq))
    has_bk = bool(np.any(bk))
    has_bv = bool(np.any(bv))
    key = (mode, has_bq, has_bk, has_bv)
    if key not in _CACHE:
        _CACHE[key] = _build(*key)
    nc = _CACHE[key]

    in_maps = [
        _prep_core(c, x, freqs_cis, mask, wq, bq, wk, bk, wv, bv, wo, mode,
                   has_bq, has_bk, has_bv)
        for c in range(8)
    ]
    res = bass_utils.run_bass_kernel_spmd(nc, in_maps, core_ids=list(range(8)), trace=trace)
    partials = np.stack([res.results[c]["out"] for c in range(8)], 0)  # (8, S, H)
    out = partials.reshape(2, 4, S, H).sum(axis=1) + bo[None, None, :]
    return out.astype(np.float32), res


def kernel(**inputs):
    out, _ = _run(inputs, trace=False)
    return out



# revision 17
# speedup vs baseline: 1.5213x; 1.5213x over previous
"""GQA attention (B=2, S=2048, H=2048, 32 heads / 8 KV groups, rope, causal-masked
softmax, output projection) distributed over 8 Trainium2 NeuronCores.

Sharding: data parallel over batch (2) x tensor parallel over KV groups (4 group-pairs).
Core c handles batch c//4 and KV groups {2*(c%4), 2*(c%4)+1} (= 8 q heads). Each core
computes its partial output projection (attn_out_shard @ wo_cols_shard.T); the host
sums the 4 partials per batch (the "all-reduce") and adds bo.

v2 (vs f32r baseline): all matmul operands in bf16 (f32r moving-operand reads cap the
PE at ~1.0 ns/row; bf16 streams at ~0.42 ns/row), causal mask applied inside the scores
PSUM accumulation via an extra [strict-lower-tri x (-8e9 * I)] matmul (frees the DVE),
AV matmuls trimmed to the causal suffix per diagonal tile, projections run in
weight-stationary passes over resident x, rope on bf16 SBUF tiles (PSUM evicted via the
otherwise-idle Act engine), softmax 1/den as Exp(-Ln(den)) on the Act engine (both live
in one activation table set; custom-DVE reciprocal_approx is broken on HW) broadcast via
a K=1 matmul, and the output projection of block qt-1 interleaved into block qt's
attention so the Act engine never sits idle. Output partials are bf16, summed in f32 on
host.
"""
import sys

for _p in ("/opt/trn_rl_repo",):
    if _p not in sys.path:
        sys.path.append(_p)

import numpy as np
import ml_dtypes

S = 2048
H = 2048
HD = 64
NQT = 4          # s_q tiles of 512
NKT = 16         # s_k tiles of 128

_CACHE = {}


def _build(mode, has_bq, has_bk, has_bv):
    import concourse.bass as bass  # noqa: F401
    import concourse.mybir as mybir
    import concourse.tile as tile
    from concourse import bacc

    f32 = mybir.dt.float32
    b16 = mybir.dt.bfloat16
    AF = mybir.ActivationFunctionType
    ALU = mybir.AluOpType

    nc = bacc.Bacc("TRN2", target_bir_lowering=False, debug=False)
    xT = nc.dram_tensor("xT", [H, S], b16, kind="ExternalInput")
    wqT = nc.dram_tensor("wqT", [H, 512], b16, kind="ExternalInput")
    wkvT = nc.dram_tensor("wkvT", [H, 256], b16, kind="ExternalInput")
    woR = nc.dram_tensor("woR", [512, H], b16, kind="ExternalInput")
    COSd = nc.dram_tensor("COSx", [128, S], b16, kind="ExternalInput")
    SINd = nc.dram_tensor("SINx", [128, S], b16, kind="ExternalInput")
    # packed constants: [L (strict lower tri) | Z (-8e9*I) | I | SEL0..SEL3]
    CSTd = nc.dram_tensor("CST", [128, 640], b16, kind="ExternalInput")
    outd = nc.dram_tensor("out", [S, H], b16, kind="ExternalOutput")
    maskd = nc.dram_tensor("maskT", [S, S], f32, kind="ExternalInput") if mode == "generic" else None
    bqd = nc.dram_tensor("bq", [512, 1], f32, kind="ExternalInput") if has_bq else None
    bkvd = nc.dram_tensor("bkv", [256, 1], f32, kind="ExternalInput") if (has_bk or has_bv) else None

    with tile.TileContext(nc) as tc:
        with (
            tc.tile_pool(name="const", bufs=1) as cstp,
            tc.tile_pool(name="wts", bufs=1) as wts,
            tc.tile_pool(name="per", bufs=1) as per,
            tc.tile_pool(name="rtmp", bufs=2) as rtp,
            tc.tile_pool(name="et", bufs=3) as etp,
            tc.tile_pool(name="rcs", bufs=2) as rcp,
            tc.tile_pool(name="avqs", bufs=2) as avp,
            tc.tile_pool(name="outs", bufs=3) as outp,
            tc.tile_pool(name="mks", bufs=2) as mkp,
        ):
            COS = cstp.tile([128, S], b16, tag="cos")
            SIN = cstp.tile([128, S], b16, tag="sin")
            CST = cstp.tile([128, 640], b16, tag="cst")
            nc.sync.dma_start(COS[:], COSd[:])
            nc.sync.dma_start(SIN[:], SINd[:])
            nc.sync.dma_start(CST[:], CSTd[:])
            Lc = CST[:, 0:128]       # L[k, m] = 1 if k < m
            Zc = CST[:, 128:256]     # -8e9 * I
            Ic = CST[:, 256:384]     # identity (transpose helper)
            # SEL[m]: all-ones row at partition 32m (recip broadcast lhsT)
            SELc = [CST[:, 384 + 64 * m:384 + 64 * (m + 1)] for m in range(4)]
            if has_bq:
                bq_t = [cstp.tile([128, 1], f32, tag=f"bq{m}", name=f"bq_t{m}") for m in range(4)]
                for m in range(4):
                    nc.sync.dma_start(bq_t[m][:], bqd[128 * m:128 * (m + 1), :])
            if has_bk or has_bv:
                bk_t = cstp.tile([128, 1], f32, tag="bkt")
                bv_t = cstp.tile([128, 1], f32, tag="bvt")
                nc.sync.dma_start(bk_t[:], bkvd[0:128, :])
                nc.sync.dma_start(bv_t[:], bkvd[128:256, :])

            # resident weights + full x (bf16)
            wkv_t = [wts.tile([128, 256], b16, tag=f"wkv{k}", name=f"wkv_t{k}") for k in range(16)]
            for k in range(16):
                nc.sync.dma_start(wkv_t[k][:], wkvT[128 * k:128 * (k + 1), :])
            x_t = [wts.tile([128, S], b16, tag=f"xt{k}", name=f"x_t{k}") for k in range(16)]
            for k in range(16):
                nc.sync.dma_start(x_t[k][:], xT[128 * k:128 * (k + 1), :])
            wq_t = [wts.tile([128, 512], b16, tag=f"wq{k}", name=f"wq_t{k}") for k in range(16)]
            for k in range(16):
                nc.sync.dma_start(wq_t[k][:], wqT[128 * k:128 * (k + 1), :])
            wo_t = [wts.tile([128, S], b16, tag=f"wor{k}", name=f"wo_t{k}") for k in range(4)]
            for k in range(4):
                nc.sync.dma_start(wo_t[k][:], woR[128 * k:128 * (k + 1), :])

            # persistent intermediates (bf16 matmul operands)
            QTrot = [per.tile([128, S], b16, tag=f"qtrot{m}", name=f"QTrot{m}") for m in range(4)]
            KTrot = per.tile([128, S], b16, tag="ktrot")
            # V with a ones column per kt-block: [g0 v64 | 1 | g1 v64 | 1] x 16 kt
            Vp = per.tile([128, 130 * NKT], b16, tag="vp")
            nc.gpsimd.memset(Vp[:], 1.0)  # ones columns at 130*kt+{64,129} survive
            VTt = [per.tile([128, 512], b16, tag=f"vtt{sc}", name=f"VTt{sc}") for sc in range(4)]

            def rope(ps, dst_col_slice, dst):
                # evict psum -> bf16 (Act), then dst = COS*qs + SIN*(block-swapped qs)
                # on all-bf16 all-SBUF DVE ops (fast DVE modes)
                ssl = dst_col_slice
                qs = rtp.tile([128, 512], b16, tag="qs")
                nc.scalar.activation(qs[:], ps[:], AF.Copy)
                t1 = rtp.tile([128, 512], b16, tag="t1")
                t2 = rtp.tile([128, 512], b16, tag="t2")
                nc.vector.tensor_tensor(t1[:], qs[:], COS[:, ssl], ALU.mult)
                for blk in range(4):
                    sb = 32 * (blk ^ 1)
                    db = 32 * blk
                    # SIN is pre-swapped on host so both SBUF inputs share base sb
                    # (verifier: equal input base partitions when both are in SB)
                    nc.vector.tensor_tensor(
                        t2[db:db + 32, :], qs[sb:sb + 32, :], SIN[sb:sb + 32, ssl], ALU.mult)
                nc.vector.tensor_tensor(dst[:, ssl], t1[:], t2[:], ALU.add)

            # ---------- Phase P: projections + rope + V transpose ----------
            with tc.tile_pool(name="psP", bufs=1, space="PSUM") as psP:
                pp = lambda i: psP.tile([128, 512], f32, tag=f"pp{i}", name=f"pp{i}")

                # pass KV: kvK -> pp0-3, kvV -> pp4-7 (weight-stationary over 4 blocks)
                kvK = [pp(i) for i in range(4)]
                kvV = [pp(4 + i) for i in range(4)]
                for k in range(16):
                    st = (k == 0)
                    sp = (k == 15)
                    for sc in range(4):
                        ssl = slice(512 * sc, 512 * (sc + 1))
                        nc.tensor.matmul(kvK[sc][:], wkv_t[k][:, 0:128], x_t[k][:, ssl], start=st, stop=sp)
                    for sc in range(4):
                        ssl = slice(512 * sc, 512 * (sc + 1))
                        nc.tensor.matmul(kvV[sc][:], wkv_t[k][:, 128:256], x_t[k][:, ssl], start=st, stop=sp)
                for sc in range(4):
                    ssl = slice(512 * sc, 512 * (sc + 1))
                    if has_bk:
                        nc.vector.tensor_scalar_add(kvK[sc][:], kvK[sc][:], bk_t[:])
                    rope(kvK[sc], ssl, KTrot)
                    if has_bv:
                        nc.vector.tensor_scalar_add(kvV[sc][:], kvV[sc][:], bv_t[:])
                    nc.scalar.activation(VTt[sc][:], kvV[sc][:], AF.Copy)
                # V transpose: 16 kt blocks via PE transpose (bf16), scatter into Vp
                for kt in range(16):
                    sc, j = divmod(kt, 4)
                    vt = psP.tile([128, 512], f32, tag=f"pp{4 + (kt % 2)}", name=f"vt{kt}")
                    vps = vt[:].bitcast(b16)[:, 0:128]
                    nc.tensor.transpose(vps, VTt[sc][:, 128 * j:128 * (j + 1)], Ic)
                    dst = Vp[:, 130 * kt:130 * kt + 130].rearrange("p (two x) -> p two x", two=2)[:, :, 0:64]
                    src_ap = vps.rearrange("p (two x) -> p two x", two=2)
                    nc.vector.tensor_copy(dst, src_ap)

                # passes Q m=0..3: alternate pp0-3 / pp4-7
                for m in range(4):
                    base = 0 if (m % 2 == 0) else 4
                    qp = [pp(base + i) for i in range(4)]
                    for k in range(16):
                        st = (k == 0)
                        sp = (k == 15)
                        for sc in range(4):
                            ssl = slice(512 * sc, 512 * (sc + 1))
                            nc.tensor.matmul(qp[sc][:], wq_t[k][:, 128 * m:128 * (m + 1)],
                                             x_t[k][:, ssl], start=st, stop=sp)
                    for sc in range(4):
                        ssl = slice(512 * sc, 512 * (sc + 1))
                        if has_bq:
                            nc.vector.tensor_scalar_add(qp[sc][:], qp[sc][:], bq_t[m][:])
                        rope(qp[sc], ssl, QTrot[m])

            # ---------- Phase A: attention, outproj(qt-1) interleaved into qt ----------
            with tc.tile_pool(name="psA", bufs=1, space="PSUM") as psA:
                pending = []   # outproj closures for the previous qt block
                prev_avq = None

                def outproj_unit(avq_all, qt, j, npair):
                    def emit():
                        mm = 4 * qt + j
                        op = psA.tile([128, 1024], f32, tag="sc", bufs=2, name="op")
                        for k in range(4):
                            for nn in range(2):
                                nsl = slice(1024 * npair + 512 * nn, 1024 * npair + 512 * (nn + 1))
                                nc.tensor.matmul(op[:, 512 * nn:512 * (nn + 1)],
                                                 avq_all[k][:, 128 * j:128 * (j + 1)],
                                                 wo_t[k][:, nsl], start=(k == 0), stop=(k == 3))
                        ot = outp.tile([128, 1024], b16, tag="ot", name="ot")
                        nc.vector.tensor_copy(ot[:], op[:])
                        nc.sync.dma_start(
                            outd[128 * mm:128 * (mm + 1), 1024 * npair:1024 * (npair + 1)], ot[:])
                    return emit

                for qt in range(NQT):
                    qb0 = 512 * qt
                    n_kt = 4 * qt + 4 if mode == "causal" else NKT
                    avq_all = [avp.tile([128, 512], b16, tag=f"avq{m}", bufs=2, name=f"avq{m}")
                               for m in range(4)]
                    den = [rcp.tile([128, 512], f32, tag=f"den{h}", name=f"den{h}") for h in range(2)]
                    for h in range(2):
                        nc.gpsimd.memset(den[h][:], 1.0)   # unused rows stay Ln/Exp-safe
                    for m in range(4):
                        av = [psA.tile([128, 512], f32, tag="av", bufs=2, name=f"av{h}") for h in range(2)]
                        for pr in range(n_kt // 2):
                            for hloc in range(2):
                                g = hloc
                                qb = 64 * hloc
                                sc2 = psA.tile([128, 1024], f32, tag="sc", bufs=2, name="sc2")
                                eT = etp.tile([128, 1024], b16, tag="eT", name="eT")
                                for half in range(2):
                                    kt = 2 * pr + half
                                    t = kt - 4 * qt
                                    ks = KTrot[64 * g:64 * g + 64, 128 * kt:128 * (kt + 1)]
                                    if mode == "causal" and t >= 0:
                                        # diagonal tile: full-width scores open the bank's
                                        # accumulation group, then the -8e9 triangle
                                        # accumulates onto the already-written block
                                        b0 = 512 * half + 128 * t
                                        nc.tensor.matmul(
                                            sc2[:, 512 * half:512 * (half + 1)], ks,
                                            QTrot[m][qb:qb + 64, qb0:qb0 + 512],
                                            start=True, stop=False)
                                        nc.tensor.matmul(sc2[:, b0:b0 + 128], Lc, Zc,
                                                         start=False, stop=True)
                                    else:
                                        nc.tensor.matmul(
                                            sc2[:, 512 * half:512 * (half + 1)], ks,
                                            QTrot[m][qb:qb + 64, qb0:qb0 + 512],
                                            start=True, stop=True)
                                if mode == "generic":
                                    for half in range(2):
                                        kt = 2 * pr + half
                                        mk = mkp.tile([128, 512], f32, tag="mk", name="mk")
                                        nc.sync.dma_start(mk[:], maskd[128 * kt:128 * (kt + 1), qb0:qb0 + 512])
                                        stt = mkp.tile([128, 512], f32, tag="stt", name="stt")
                                        nc.vector.scalar_tensor_tensor(
                                            stt[:], sc2[:, 512 * half:512 * (half + 1)], 0.125, mk[:],
                                            ALU.mult, ALU.add)
                                        nc.scalar.activation(
                                            eT[:, 512 * half:512 * (half + 1)], stt[:], AF.Exp, scale=1.0)
                                else:
                                    nc.scalar.activation(eT[:], sc2[:], AF.Exp, scale=0.125)
                                for half in range(2):
                                    kt = 2 * pr + half
                                    t = kt - 4 * qt
                                    lo = 128 * t if (mode == "causal" and t >= 0) else 0
                                    nc.tensor.matmul(
                                        av[hloc][0:65, lo:512],
                                        Vp[:, 130 * kt + 65 * g:130 * kt + 65 * g + 65],
                                        eT[:, 512 * half + lo:512 * (half + 1)],
                                        start=(kt == 0), stop=(kt == n_kt - 1))
                        for hloc in range(2):
                            qb = 64 * hloc
                            # stage denominator row at partition 32m; evict unnormalized AV
                            nc.vector.tensor_scalar_mul(den[hloc][32 * m:32 * m + 1, :],
                                                        av[hloc][64:65, :], 1.0)
                            nc.vector.tensor_scalar_mul(avq_all[m][qb:qb + 64, :],
                                                        av[hloc][0:64, :], 1.0)
                        # drain 2 outproj units of the previous qt block
                        for _ in range(2):
                            if pending:
                                pending.pop(0)()
                    # batched 1/den via Act: recipB = Exp(-Ln(den)), written as bf16
                    recipB = []
                    for h in range(2):
                        lnT = rcp.tile([128, 512], f32, tag=f"ln{h}", name=f"ln{h}")
                        nc.scalar.activation(lnT[:], den[h][:], AF.Ln)
                        rb = rcp.tile([128, 512], b16, tag=f"rb{h}", name=f"rb{h}")
                        nc.scalar.activation(rb[:], lnT[:], AF.Exp, scale=-1.0)
                        recipB.append(rb)
                    for m in range(4):
                        for hloc in range(2):
                            qb = 64 * hloc
                            rcb = psA.tile([128, 512], f32, tag="rcb", bufs=2, name="rcb")
                            nc.tensor.matmul(rcb[0:64, :], SELc[m],
                                             recipB[hloc][:], start=True, stop=True)
                            nc.vector.tensor_tensor(avq_all[m][qb:qb + 64, :],
                                                    avq_all[m][qb:qb + 64, :],
                                                    rcb[0:64, :], ALU.mult)
                    pending = [outproj_unit(avq_all, qt, j, npair)
                               for j in range(4) for npair in range(2)]
                for emit in pending:
                    emit()

    nc.compile()
    return nc


_PERM64 = np.concatenate([np.arange(0, 64, 2), np.arange(1, 64, 2)])
# Q-tile m holds local heads (m, m+4) so each head's partition base (0/64) matches
# its KV group's base in KTrot (group g at rows 64g) - matmul requires equal bases.
_HEADS_ORDER = np.array([0, 4, 1, 5, 2, 6, 3, 7])
_BF16 = ml_dtypes.bfloat16


def _prep_core(c, x, freqs_cis, mask, wq, bq, wk, bk, wv, bv, wo, mode,
               has_bq, has_bk, has_bv):
    b, gp = divmod(c, 4)
    f = np.float32
    xT = np.ascontiguousarray(x[b].T.astype(_BF16))
    wq_c = wq[512 * gp:512 * (gp + 1)].reshape(8, 64, H)[_HEADS_ORDER][:, _PERM64, :].reshape(512, H)
    wqT = np.ascontiguousarray(wq_c.T.astype(_BF16))
    wk_c = wk[128 * gp:128 * (gp + 1)].reshape(2, 64, H)[:, _PERM64, :].reshape(128, H)
    wv_c = wv[128 * gp:128 * (gp + 1)]
    wkvT = np.ascontiguousarray(np.concatenate([wk_c, wv_c], 0).T.astype(_BF16))
    woR = wo[:, 512 * gp:512 * (gp + 1)].T.reshape(8, 64, H)[_HEADS_ORDER].reshape(512, H)
    woR = np.ascontiguousarray(woR.astype(_BF16))
    cosT = np.ascontiguousarray(freqs_cis[:, 0::2].T, dtype=f)   # (32, S)
    sinT = np.ascontiguousarray(freqs_cis[:, 1::2].T, dtype=f)
    COS = np.tile(cosT, (4, 1)).astype(_BF16)
    # partition blocks pre-swapped pairwise: block at base sb holds the sign-applied
    # sin coefficients of destination block db = sb ^ 32 (see rope in _build)
    SIN = np.concatenate([sinT, -sinT, sinT, -sinT], 0).astype(_BF16)
    i = np.arange(128)
    L = (i[:, None] < i[None, :]).astype(_BF16)          # L[k, m] = 1 if k < m
    Z = (np.float32(-8e9) * np.eye(128, dtype=f)).astype(_BF16)
    I = np.eye(128, dtype=f).astype(_BF16)
    SEL = []
    for m4 in range(4):
        s = np.zeros((128, 64), dtype=_BF16)
        s[32 * m4, :] = 1
        SEL.append(s)
    CST = np.concatenate([L, Z, I] + SEL, axis=1)
    m = {"xT": xT, "wqT": wqT, "wkvT": wkvT, "woR": woR,
         "COSx": np.ascontiguousarray(COS), "SINx": np.ascontiguousarray(SIN),
         "CST": np.ascontiguousarray(CST)}
    if mode == "generic":
        m["maskT"] = np.ascontiguousarray(mask.T, dtype=f)
    if has_bq:
        bq_c = bq[512 * gp:512 * (gp + 1)].reshape(8, 64)[_HEADS_ORDER][:, _PERM64].reshape(512, 1)
        m["bq"] = np.ascontiguousarray(bq_c, dtype=f)
    if has_bk or has_bv:
        bk_c = bk[128 * gp:128 * (gp + 1)].reshape(2, 64)[:, _PERM64].reshape(128)
        bv_c = bv[128 * gp:128 * (gp + 1)]
        m["bkv"] = np.ascontiguousarray(np.concatenate([bk_c, bv_c]).reshape(256, 1), dtype=f)
    return m


def _detect_mode(mask):
    causal = np.where(np.tril(np.ones((S, S), dtype=bool)), np.float32(0.0), np.float32(-1e9))
    if np.array_equal(mask, causal):
        return "causal"
    if not np.any(mask):
        return "zeros"
    return "generic"


def _run(inputs, trace=False):
    from concourse import bass_utils
    x = np.asarray(inputs["x"], dtype=np.float32)
    freqs_cis = np.asarray(inputs["freqs_cis"], dtype=np.float32)
    mask = np.asarray(inputs["mask"], dtype=np.float32)
    wq = np.asarray(inputs["wq"], dtype=np.float32)
    bq = np.asarray(inputs["bq"], dtype=np.float32)
    wk = np.asarray(inputs["wk"], dtype=np.float32)
    bk = np.asarray(inputs["bk"], dtype=np.float32)
    wv = np.asarray(inputs["wv"], dtype=np.float32)
    bv = np.asarray(inputs["bv"], dtype=np.float32)
    wo = np.asarray(inputs["wo"], dtype=np.float32)
    bo = np.asarray(inputs["bo"], dtype=np.float32)

    mode = _detect_mode(mask)
    has_bq = bool(np.any(bq))
    has_bk = bool(np.any(bk))
    has_bv = bool(np.any(bv))
    key = (mode, has_bq, has_bk, has_bv)
    if key not in _CACHE:
        _CACHE[key] = _build(*key)
    nc = _CACHE[key]

    in_maps = [
        _prep_core(c, x, freqs_cis, mask, wq, bq, wk, bk, wv, bv, wo, mode,
                   has_bq, has_bk, has_bv)
        for c in range(8)
    ]
    res = bass_utils.run_bass_kernel_spmd(nc, in_maps, core_ids=list(range(8)), trace=trace)
    partials = np.stack([res.results[c]["out"].astype(np.float32) for c in range(8)], 0)
    out = partials.reshape(2, 4, S, H).sum(axis=1) + bo[None, None, :]
    return out.astype(np.float32), res


def kernel(**inputs):
    out, _ = _run(inputs, trace=False)
    return out
